# revision 1
# baseline (speedup 1.0000x reference)
"""Trainium2 Bass kernel for nn_MultiHeadAttention (B=2, S=2048, D=1024, H=16).

Reference semantics (note the *raw-view* head split):
    q = query @ Wq.T + bq                  # [B, S, D]
    q = q.reshape(B, H, S, DK)             # raw view: head h = rows [h*128,(h+1)*128)
    scores = q @ k.T / sqrt(DK), causal mask, softmax
    ctx    = softmax @ v                   # [B, H, S, DK]
    out    = ctx.transpose(0,2,1,3).reshape(B,S,D) @ Wo.T + bo

Sharding: 8 cores = 2 batches x 4 head-groups.  Core (b, g) owns heads
[4g, 4g+4) of batch b = rows [512g, 512g+512) of the QKV projections.  Each
core computes its 4 heads' attention plus its partial out-projection
C_heads @ Wo[:, head cols].T; the host sums 4 partials per batch + bo.

v2 design (vs v1): bf16 end-to-end (f32 psum accumulate), one batched DMA
per tensor instead of per-tile DMAs, v-reshape as one Pool-engine DMA per
head, pair-split k projection so pair-0 attention starts while pair-1 k/v
still project (as PE fillers under ACT-bound exp), width-trimmed diagonal
score/ctx matmuls, ACT reserved for exp during attention (copies on DVE,
DMA issue on SP/Pool only).
"""

import os
import sys

import numpy as np

_TRN_REPO = "/opt/trn_rl_repo"
if _TRN_REPO not in sys.path:
    sys.path.insert(0, _TRN_REPO)

B, S, D, H = 2, 2048, 1024, 16
DK = D // H  # 64
N_CORES = 8
HEADS_PER_CORE = 4
ROWS_PER_CORE = 512  # rows of the projection output owned per core
QW = 512  # q-position window (psum free-dim)
KT = 128  # k-position tile


def _build_program(repeat=1, phases=3):
    import concourse.bass as bass
    import concourse.bacc as bacc
    import concourse.mybir as mybir
    from concourse.tile import TileContext
    from contextlib import ExitStack

    f32 = mybir.dt.float32
    bf16 = mybir.dt.bfloat16
    f8 = mybir.dt.float8e4
    DRow = mybir.MatmulPerfMode.DoubleRow
    Exp = mybir.ActivationFunctionType.Exp
    Identity = mybir.ActivationFunctionType.Identity
    MUL = mybir.AluOpType.mult
    ADD = mybir.AluOpType.add

    nc = bacc.Bacc("TRN2", target_bir_lowering=False, debug=False)

    # ---- DRAM parameters (host pre-tiled / pre-transposed, bf16) ----
    xq = nc.dram_tensor("xq", [8, 128, QW], bf16, kind="ExternalInput")
    xk = nc.dram_tensor("xk", [8, 128, QW], bf16, kind="ExternalInput")
    xv = nc.dram_tensor("xv", [8, 128, QW], bf16, kind="ExternalInput")
    wq = nc.dram_tensor("wq", [2, 8, 128, 512], bf16, kind="ExternalInput")
    wk = nc.dram_tensor("wk", [2, 8, 128, 512], bf16, kind="ExternalInput")
    wv = nc.dram_tensor("wv", [2, 8, 128, 512], bf16, kind="ExternalInput")
    wo = nc.dram_tensor("wo", [2, 128, 1024], bf16, kind="ExternalInput")
    bqd = nc.dram_tensor("bqd", [128, 16], f32, kind="ExternalInput")
    bkd = nc.dram_tensor("bkd", [128, 16], f32, kind="ExternalInput")
    bvr = nc.dram_tensor("bvr", [1, 1024], bf16, kind="ExternalInput")
    tri = nc.dram_tensor("tri", [128, 256], bf16, kind="ExternalInput")
    ones128 = nc.dram_tensor("ones128", [1, 128], bf16, kind="ExternalInput")
    out = nc.dram_tensor("out", [S, D], bf16, kind="ExternalOutput")
    # per-head DRAM scratch for the v reshape round-trip (vstage layout)
    vscr = [
        nc.dram_tensor(f"vscr{h}", [128, 1024], bf16, kind="Internal")
        for h in range(4)
    ]

    with TileContext(nc) as tc:
      with ExitStack() as stack:
        persist = stack.enter_context(tc.tile_pool(name="persist", bufs=1))
        vhp = stack.enter_context(tc.tile_pool(name="vhp", bufs=1))
        small = stack.enter_context(tc.tile_pool(name="small", bufs=12))
        xp = stack.enter_context(tc.tile_pool(name="xp", bufs=3))
        wp = stack.enter_context(tc.tile_pool(name="wp", bufs=3))
        vsb = stack.enter_context(tc.tile_pool(name="vsb", bufs=1))
        ptp = stack.enter_context(tc.tile_pool(name="ptp", bufs=8))
        ptp8 = stack.enter_context(tc.tile_pool(name="ptp8", bufs=3))
        vh8p = stack.enter_context(tc.tile_pool(name="vh8p", bufs=1))
        wop = stack.enter_context(tc.tile_pool(name="wop", bufs=1))
        osb = stack.enter_context(tc.tile_pool(name="osb", bufs=5))
        for rep in range(repeat):
            # persistent tiles
            qpair = [persist.tile([128, S], bf16, tag=f"qpair{p}", name=f"qpair{p}") for p in range(2)]
            kpair = [persist.tile([128, S], bf16, tag=f"kpair{p}", name=f"kpair{p}") for p in range(2)]
            ctxT = [persist.tile([128, S], bf16, tag=f"ctxT{p}", name=f"ctxT{p}") for p in range(2)]
            tri01 = persist.tile([128, 256], bf16, tag="tri01")
            bq_t = persist.tile([128, 16], f32, tag="bq_t")
            bk_t = persist.tile([128, 16], f32, tag="bk_t")
            bv_t = persist.tile([1, 1024], bf16, tag="bv_t")
            ones_row = persist.tile([1, 128], bf16, tag="ones_row")
            # consts on the ACT queue (idle at start); inputs stream on SP
            nc.scalar.dma_start(out=tri01[:], in_=tri[:])
            nc.scalar.dma_start(out=bq_t[:], in_=bqd[:])
            nc.scalar.dma_start(out=bk_t[:], in_=bkd[:])
            nc.scalar.dma_start(out=bv_t[:], in_=bvr[:])
            nc.scalar.dma_start(out=ones_row[:], in_=ones128[:])

            # vh: one tile per head [128 kpos, 16*(DK+1)]; per ktile j cols
            # [j*65, j*65+64) = v data, col j*65+64 = ones (softmax denom)
            vh = [
                vhp.tile([128, 16 * (DK + 1)], bf16, tag=f"vh_{h}", name=f"vh_{h}")
                for h in range(4)
            ]
            vh8 = [
                vh8p.tile([128, 16 * 128], f8, tag=f"vh8_{h}", name=f"vh8_{h}")
                for h in range(4)
            ]
            vstage = [
                vsb.tile([128, 1024], bf16, tag=f"vst_{rt}", name=f"vst_{rt}")
                for rt in range(4)
            ]

            # batched input DMAs (SP queue, in consumption order);
            # partition-major iteration on both sides so the sbuf AP's dim0
            # is the 128-partition dim (full DMA-lane parallelism)
            def load_x(dram_t, nm, eng):
                t = xp.tile([128, 8 * QW], bf16, tag="xall", name=nm)
                eng.dma_start(
                    out=t[:].rearrange("p (i r) -> p i r", i=8),
                    in_=dram_t[:].rearrange("i p r -> p i r"),
                )
                return t

            def load_w(dram_t, nm, eng):
                t = wp.tile([128, 2 * 8 * 512], bf16, tag="wall", name=nm)
                for fh in range(2):
                    eng.dma_start(
                        out=t[:, fh * 4096 : (fh + 1) * 4096].rearrange(
                            "p (i r) -> p i r", i=8
                        ),
                        in_=dram_t[fh].rearrange("i p r -> p i r"),
                    )
                return t

            # q operands fine-grained + interleaved on SP so the first
            # projection matmuls start ~4.5us in; the rest streams on Pool
            # (SWDGE) whose consumers run later.
            xq_t = xp.tile([128, 8 * QW], bf16, tag="xall", name="xq_t")
            wq_t = wp.tile([128, 2 * 8 * 512], bf16, tag="wall", name="wq_t")

            def xq_piece(ih):  # i in [2*ih, 2*ih+2)
                nc.sync.dma_start(
                    out=xq_t[:, ih * 1024 : (ih + 1) * 1024].rearrange(
                        "p (i r) -> p i r", i=2
                    ),
                    in_=xq[2 * ih : 2 * ih + 2].rearrange("i p r -> p i r"),
                )

            def wq_piece(fh, ih):  # i in [4*ih, 4*ih+4)
                nc.sync.dma_start(
                    out=wq_t[:, fh * 4096 + ih * 2048 : fh * 4096 + (ih + 1) * 2048]
                    .rearrange("p (i r) -> p i r", i=4),
                    in_=wq[fh, 4 * ih : 4 * ih + 4].rearrange("i p r -> p i r"),
                )

            def wq_fine(fh, i0, i1):  # i in [i0, i1)
                nc.sync.dma_start(
                    out=wq_t[:, fh * 4096 + i0 * 512 : fh * 4096 + i1 * 512]
                    .rearrange("p (i r) -> p i r", i=i1 - i0),
                    in_=wq[fh, i0:i1].rearrange("i p r -> p i r"),
                )

            def xq_fine(i0, i1):
                nc.sync.dma_start(
                    out=xq_t[:, i0 * 512 : i1 * 512].rearrange(
                        "p (i r) -> p i r", i=i1 - i0
                    ),
                    in_=xq[i0:i1].rearrange("i p r -> p i r"),
                )

            # smallest pieces first: the i=0 matmul of the first q psum can
            # start ~2us in instead of waiting for full-tensor transfers
            wq_fine(0, 0, 1)
            xq_fine(0, 1)
            wq_fine(0, 1, 4)
            xq_fine(1, 4)
            wq_fine(0, 4, 8)
            xq_fine(4, 8)
            wq_fine(1, 0, 8)
            xv_t = load_x(xv, "xv_t", nc.gpsimd)
            wv_t = load_w(wv, "wv_t", nc.gpsimd)
            xk_t = load_x(xk, "xk_t", nc.gpsimd)
            wk_t = load_w(wk, "wk_t", nc.gpsimd)
            wo_t = wop.tile([128, 2048], bf16, tag="wo", name="wo_t")
            nc.sync.dma_start(
                out=wo_t[:].rearrange("p (a o) -> p a o", a=2),
                in_=wo[:].rearrange("a p o -> p a o"),
            )

            # ones columns of vh (memset once per rep)
            for rt in range(4):
                nc.vector.memset(
                    vh[rt][:].rearrange("p (j e) -> p j e", e=65)[:, :, 64], 1.0
                )

            # ---------------- Phase P: projections ----------------
            scatter_flip = [0]

            def scatter(ps, f, heads, cs, engines=("v", "a")):
                # psum [128 f, w r] covering local rows [cs, cs+w) -> q/k pair
                # tiles, strided; bias added via per-partition scalar ptr
                for c2 in range(2):
                    chunk = 2 * f + c2
                    for h in heads:
                        dst = (
                            dest_pair[h // 2][(h % 2) * 64 : (h % 2) * 64 + 64, :]
                            .rearrange("p (r c) -> p r c", c=16)[:, :, chunk]
                        )
                        src = ps[c2 * 64 : (c2 + 1) * 64, h * 128 - cs : h * 128 - cs + 128]
                        bias_ap = bias_t[c2 * 64 : (c2 + 1) * 64, chunk : chunk + 1]
                        eng = engines[scatter_flip[0] % len(engines)]
                        scatter_flip[0] += 1
                        if eng == "v":
                            nc.vector.tensor_scalar(
                                out=dst, in0=src, scalar1=bias_ap, scalar2=None, op0=ADD
                            )
                        else:
                            nc.scalar.activation(dst, src, Identity, bias=bias_ap)

            def emit_v_group(rt, fh, pool):
                ps = pool.tile([128, 512], f32, tag="fill", name="vproj")
                for i in range(8):
                    nc.tensor.matmul(
                        ps[:],
                        xv_t[:, i * 512 + rt * 128 : i * 512 + (rt + 1) * 128],
                        wv_t[:, (fh * 8 + i) * 512 : (fh * 8 + i + 1) * 512],
                        start=(i == 0),
                        stop=False,
                    )
                nc.tensor.matmul(
                    ps[:],
                    ones_row[:],
                    bv_t[:, fh * 512 : (fh + 1) * 512],
                    start=False,
                    stop=True,
                )
                nc.vector.tensor_copy(
                    out=vstage[rt][:, fh * 512 : (fh + 1) * 512], in_=ps[:]
                )

            def emit_v_reshape(rt):
                # DRAM round-trip (both hops partition-major = cheap):
                # vstage[rt] -> vscr[rt] (plain), then the reshape on the
                # dram side -> vh[rt] [p'=(rr c), (j d)], ones cols skipped
                nc.gpsimd.dma_start(out=vscr[rt][:], in_=vstage[rt][:])
                nc.gpsimd.dma_start(
                    out=vh[rt][:].rearrange("p (j e) -> p j e", e=65)[:, :, 0:64],
                    in_=vscr[rt][:].rearrange("(j r) (c d) -> r c j d", r=8, d=64),
                )
                # fp8 copy (padded 128-col blocks) for DoubleRow ctx
                nc.vector.tensor_copy(
                    out=vh8[rt][:].rearrange("p (j e) -> p j e", e=128)[:, :, 0:65],
                    in_=vh[rt][:].rearrange("p (j e) -> p j e", e=65),
                )

            with tc.tile_pool(name=f"ppsP{rep}", bufs=3, space="PSUM") as ppsP:
                # q projection: 8 full-width psums
                dest_pair = qpair
                bias_t = bq_t
                for fh in range(2):
                    for f4 in range(4):
                        f = fh * 4 + f4
                        ps = ppsP.tile([128, QW], f32, tag="proj")
                        for i in range(8):
                            nc.tensor.matmul(
                                ps[:],
                                wq_t[:, (fh * 8 + i) * 512 + f4 * 128 : (fh * 8 + i) * 512 + (f4 + 1) * 128],
                                xq_t[:, i * 512 : (i + 1) * 512],
                                start=(i == 0),
                                stop=(i == 7),
                            )
                        scatter(ps, f, heads=(0, 1, 2, 3), cs=0)

                # v rt 0,1 (pair-0 heads) before attention
                for rt in range(2):
                    for fh in range(2):
                        emit_v_group(rt, fh, ppsP)
                    emit_v_reshape(rt)

                # k projection pair-0 half: 8 psums [128, 256]
                dest_pair = kpair
                bias_t = bk_t
                for fh in range(2):
                    for f4 in range(4):
                        f = fh * 4 + f4
                        ps = ppsP.tile([128, QW], f32, tag="proj")
                        for i in range(8):
                            nc.tensor.matmul(
                                ps[:, 0:256],
                                wk_t[:, (fh * 8 + i) * 512 + f4 * 128 : (fh * 8 + i) * 512 + (f4 + 1) * 128],
                                xk_t[:, i * 512 : i * 512 + 256],
                                start=(i == 0),
                                stop=(i == 7),
                            )
                        scatter(ps, f, heads=(0, 1), cs=0)

            if phases < 3:
                # debug: stop after phase P (emit k1 + v23 inline, dump qpair)
                with tc.tile_pool(name=f"dbg{rep}", bufs=3, space="PSUM") as dps:
                    kpair_d, bias_d = kpair, bk_t
                    for fh in range(2):
                        for f4 in range(4):
                            f = fh * 4 + f4
                            ps = dps.tile([128, QW], f32, tag="proj")
                            for i in range(8):
                                nc.tensor.matmul(
                                    ps[:, 0:256],
                                    wk_t[:, (fh * 8 + i) * 512 + f4 * 128 : (fh * 8 + i) * 512 + (f4 + 1) * 128],
                                    xk_t[:, i * 512 + 256 : i * 512 + 512],
                                    start=(i == 0),
                                    stop=(i == 7),
                                )
                            dest_pair = kpair
                            bias_t = bk_t
                            scatter(ps, f, heads=(2, 3), cs=256)
                    for rt in range(2, 4):
                        for fh in range(2):
                            emit_v_group(rt, fh, dps)
                        emit_v_reshape(rt)
                nc.sync.dma_start(
                    out=out[0:128, :], in_=qpair[0][:, 0:1024]
                )
                continue

            # preload Exp table while PE finishes phase P
            dummy = small.tile([1, 4], f32, tag="dummy")
            nc.scalar.activation(dummy[:], tri01[0:1, 0:4], Exp)

            # ---------------- Phase A: attention ----------------
            with (
                tc.tile_pool(name=f"scps{rep}", bufs=2, space="PSUM") as scps,
                tc.tile_pool(name=f"ctxps{rep}", bufs=2, space="PSUM") as ctxps,
                tc.tile_pool(name=f"fps{rep}", bufs=2, space="PSUM") as fps,
            ):
                fillers = []

                # pair-1 k projection + scatters (DVE only; ACT is exp-bound)
                def emit_k1_group(fh, f4):
                    f = fh * 4 + f4
                    ps = fps.tile([128, 512], f32, tag="fill", name="k1proj")
                    for i in range(8):
                        nc.tensor.matmul(
                            ps[:, 0:256],
                            wk_t[:, (fh * 8 + i) * 512 + f4 * 128 : (fh * 8 + i) * 512 + (f4 + 1) * 128],
                            xk_t[:, i * 512 + 256 : i * 512 + 512],
                            start=(i == 0),
                            stop=(i == 7),
                        )
                    nonlocal_dest = kpair
                    for c2 in range(2):
                        chunk = 2 * f + c2
                        for h in (2, 3):
                            dst = (
                                nonlocal_dest[h // 2][(h % 2) * 64 : (h % 2) * 64 + 64, :]
                                .rearrange("p (r c) -> p r c", c=16)[:, :, chunk]
                            )
                            src = ps[c2 * 64 : (c2 + 1) * 64, h * 128 - 256 : h * 128 - 256 + 128]
                            bias_ap = bk_t[c2 * 64 : (c2 + 1) * 64, chunk : chunk + 1]
                            nc.vector.tensor_scalar(
                                out=dst, in0=src, scalar1=bias_ap, scalar2=None, op0=ADD
                            )

                def emit_v_group_f(rt, fh):
                    emit_v_group(rt, fh, fps)

                for fh in range(2):
                    for f4 in range(4):
                        fillers.append(lambda fh=fh, f4=f4: emit_k1_group(fh, f4))
                for rt in range(2, 4):
                    for fh in range(2):
                        fillers.append(lambda rt=rt, fh=fh: emit_v_group_f(rt, fh))
                    fillers.append(lambda rt=rt: emit_v_reshape(rt))

                emitted_st = set()

                def emit_out_stile(st, pool=None, alt_copy=False):
                    emitted_st.add(st)
                    ostage = osb.tile([128, 1024], bf16, tag="ostage", name="ostage")
                    for og in range(2):
                        ps = (pool or fps).tile([128, 512], f32, tag="fill", name="ops")
                        for pair in range(2):
                            nc.tensor.matmul(
                                ps[:],
                                ctxT[pair][:, st * 128 : (st + 1) * 128],
                                wo_t[:, pair * 1024 + og * 512 : pair * 1024 + (og + 1) * 512],
                                start=(pair == 0),
                                stop=(pair == 1),
                            )
                        if alt_copy and og == 1:
                            nc.scalar.activation(
                                ostage[:, og * 512 : (og + 1) * 512], ps[:], Identity
                            )
                        else:
                            nc.vector.tensor_copy(
                                out=ostage[:, og * 512 : (og + 1) * 512], in_=ps[:]
                            )
                    nc.sync.dma_start(
                        out=out[st * 128 : (st + 1) * 128, :], in_=ostage[:]
                    )

                def scores(pair, qi, kj):
                    d = kj - 4 * qi
                    off = max(0, 128 * d)
                    sp = scps.tile([128, 2 * QW], f32, tag="sduo")
                    for h2 in range(2):
                        nc.tensor.matmul(
                            sp[:, h2 * QW + off : (h2 + 1) * QW],
                            kpair[pair][h2 * 64 : h2 * 64 + 64, kj * KT : (kj + 1) * KT],
                            qpair[pair][h2 * 64 : h2 * 64 + 64, qi * QW + off : (qi + 1) * QW],
                            start=True,
                            stop=True,
                        )
                    return sp

                steps = []
                for pair in range(2):
                    for qi in range(4):
                        nkt = 4 * qi + 4
                        for kj in range(nkt):
                            steps.append((pair, qi, kj, nkt))

                # depth-2 score prefetch: 3 psum duos keep the PE two steps
                # ahead of the ACT exp so per-step handoff latency is hidden
                cps_map = {}
                pt8_map = {}
                flushed = [False]

                def emit_scores(idx):
                    if idx >= len(steps):
                        return None
                    if steps[idx][0] == 1 and not flushed[0]:
                        # flush: kpair[1]/vh[2,3] must be emitted before
                        # pair-1 scores/ctx enter the PE queue
                        while fillers:
                            fillers.pop(0)()
                        flushed[0] = True
                    return scores(*steps[idx][:3])

                squeue = [emit_scores(0)]
                for si, (pair, qi, kj, nkt) in enumerate(steps):
                    s_cur = squeue.pop(0)
                    squeue.append(emit_scores(si + 1))
                    if kj == 0:
                        cps_map[(pair, qi)] = [
                            ctxps.tile([DK + 1, QW], f32, tag=f"ctx{h2}",
                                       name=f"ctx{h2}", bufs=1)
                            for h2 in range(2)
                        ]
                    cps = cps_map[(pair, qi)]
                    d = kj - 4 * qi
                    off = max(0, 128 * d)
                    if d < 0:
                        # off-diagonal: exp into one half of a paired fp8
                        # tile; the pair fuses into ONE DoubleRow ctx matmul
                        # at the odd step (accuracy-safe off the diagonal)
                        e = kj & 1
                        if e == 0:
                            pt8_map[(pair, qi)] = ptp8.tile(
                                [128, 2048], f8, tag="pt8", name="pt8"
                            )
                        pt8 = pt8_map[(pair, qi)]
                        nc.scalar.activation(
                            pt8[:, e * 1024 : (e + 1) * 1024], s_cur[:], Exp
                        )
                        if e == 1:
                            r3 = pt8[:].rearrange("p (t h x) -> p h t x", t=2, h=2)
                            for h2 in range(2):
                                h = 2 * pair + h2
                                nc.tensor.matmul(
                                    cps[h2][:],
                                    vh8[h][:, (kj - 1) * 128 : (kj + 1) * 128]
                                    .rearrange("p (t m) -> p t m", t=2)[:, :, 0:65],
                                    r3[:, h2],
                                    start=(kj == 1),
                                    stop=False,
                                    perf_mode=DRow,
                                )
                    else:
                        pt = ptp.tile([128, 2 * QW], bf16, tag="ptduo")
                        s3 = s_cur[:].rearrange("p (h x) -> p h x", h=2)[:, :, off:]
                        p3 = pt[:].rearrange("p (h x) -> p h x", h=2)[:, :, off:]
                        nc.scalar.activation(p3, s3, Exp)
                        # both heads' diagonal masks in one DVE instr
                        # (tri01 holds two side-by-side copies of the mask)
                        mv = pt[:].rearrange("p (h x) -> p h x", h=2)[
                            :, :, off : off + 128
                        ]
                        nc.vector.tensor_tensor(
                            out=mv,
                            in0=mv,
                            in1=tri01[:].rearrange("p (h x) -> p h x", h=2),
                            op=MUL,
                        )
                    if d >= 0:
                        for h2 in range(2):
                            h = 2 * pair + h2
                            nc.tensor.matmul(
                                cps[h2][:, off:] if off else cps[h2][:],
                                vh[h][:, kj * 65 : kj * 65 + 65],
                                pt[:, h2 * QW + off : (h2 + 1) * QW],
                                start=(kj == 0),
                                stop=(kj == nkt - 1),
                            )
                    if kj == nkt - 1:
                        for h2 in range(2):
                            rec = small.tile([1, QW], f32, tag="rec")
                            nc.vector.reciprocal(rec[:], cps[h2][64:65, :])
                            bc = small.tile([64, QW], f32, tag="bc")
                            nc.gpsimd.partition_broadcast(bc[:], rec[:], channels=64)
                            nc.vector.tensor_tensor(
                                out=ctxT[pair][
                                    h2 * 64 : h2 * 64 + 64, qi * QW : (qi + 1) * QW
                                ],
                                in0=cps[h2][0:64, :],
                                in1=bc[:],
                                op=MUL,
                            )
                        del cps_map[(pair, qi)]
                        if pair == 1 and qi < 3:
                            # split each stile into og-halves: finer filler
                            # granularity keeps PE fed during the ACT-bound
                            # stretch
                            ost_map = {}

                            def stile_og(st, og):
                                if og == 0:
                                    ost_map[st] = osb.tile(
                                        [128, 1024], bf16, tag="ostage", name="ostage"
                                    )
                                    emitted_st.add(st)
                                ostage = ost_map[st]
                                ps = fps.tile([128, 512], f32, tag="fill", name="ops")
                                for pr in range(2):
                                    nc.tensor.matmul(
                                        ps[:],
                                        ctxT[pr][:, st * 128 : (st + 1) * 128],
                                        wo_t[:, pr * 1024 + og * 512 : pr * 1024 + (og + 1) * 512],
                                        start=(pr == 0),
                                        stop=(pr == 1),
                                    )
                                nc.vector.tensor_copy(
                                    out=ostage[:, og * 512 : (og + 1) * 512], in_=ps[:]
                                )
                                if og == 1:
                                    nc.sync.dma_start(
                                        out=out[st * 128 : (st + 1) * 128, :],
                                        in_=ostage[:],
                                    )
                                    del ost_map[st]

                            for st in range(qi * 4, qi * 4 + 4):
                                fillers.append(lambda st=st: stile_og(st, 0))
                                fillers.append(lambda st=st: stile_og(st, 1))
                    # pop fillers at step END: their DVE copies land after
                    # this step's mask/normalize in the in-order DVE queue,
                    # keeping the exp->mask->ctx critical path unobstructed
                    if si % 2 == 1 and fillers:
                        fillers.pop(0)()
                while fillers:
                    fillers.pop(0)()

            # tail: remaining out stiles in a fresh triple-buffered psum
            # scope (attention pools closed -> banks free), copies split
            # DVE/ACT so the stile chain pipelines
            with tc.tile_pool(name=f"tps{rep}", bufs=3, space="PSUM") as tps:
                for st in range(16):
                    if st not in emitted_st:
                        emit_out_stile(st, pool=tps, alt_copy=True)

    nc.finalize()
    return nc


_NC_CACHE = {}


def _get_program(repeat=1):
    phases = int(os.environ.get("KERNEL_PHASES", "3"))
    key = (repeat, phases)
    if key not in _NC_CACHE:
        _NC_CACHE[key] = _build_program(repeat, phases)
    return _NC_CACHE[key]


def _host_inputs(query, key, value, Wq, bq, Wk, bk, Wv, bv, Wo):
    """Build the 8 per-core input maps (numpy, host-side shard/transpose)."""
    import ml_dtypes

    bf16 = ml_dtypes.bfloat16
    query = np.asarray(query, dtype=np.float32)
    key = np.asarray(key, dtype=np.float32)
    value = np.asarray(value, dtype=np.float32)
    Wq = np.asarray(Wq, dtype=np.float32)
    Wk = np.asarray(Wk, dtype=np.float32)
    Wv = np.asarray(Wv, dtype=np.float32)
    Wo = np.asarray(Wo, dtype=np.float32)
    bq = np.asarray(bq, dtype=np.float32)
    bk = np.asarray(bk, dtype=np.float32)
    bv = np.asarray(bv, dtype=np.float32)

    scale = 1.0 / np.sqrt(np.float32(DK))

    def wtile(WT):  # [1024 i, 1024 f] -> [2, 8, 128, 512] (f-half, i-tile)
        return np.ascontiguousarray(
            WT.reshape(8, 128, 2, 512).transpose(2, 0, 1, 3)
        ).astype(bf16)

    wq4 = wtile(Wq.T * scale)
    wk4 = wtile(Wk.T)
    wv4 = wtile(Wv.T)
    WoT = np.ascontiguousarray(Wo.T)  # [i, o]

    def dup_bias(b):  # [1024] -> [128, 16] dup layout
        m = b.reshape(16, 64).T  # [64, 16]
        return np.ascontiguousarray(np.vstack([m, m]))

    bqd = dup_bias(bq * scale)
    bkd = dup_bias(bk)
    bvr = bv.reshape(1, 1024).astype(bf16)
    t1 = np.triu(np.ones((128, 128), np.float32))
    tri01 = np.ascontiguousarray(np.concatenate([t1, t1], axis=1)).astype(bf16)

    in_maps = []
    for core in range(N_CORES):
        b, g = divmod(core, 4)
        sl = slice(g * ROWS_PER_CORE, (g + 1) * ROWS_PER_CORE)
        xq_ = np.ascontiguousarray(query[b, sl, :].T).astype(bf16).reshape(8, 128, QW)
        xk_ = np.ascontiguousarray(key[b, sl, :].T).astype(bf16).reshape(8, 128, QW)
        xv_ = np.ascontiguousarray(value[b, sl, :].T).astype(bf16).reshape(8, 128, QW)
        wo4 = np.ascontiguousarray(WoT[g * 256 : (g + 1) * 256, :]).astype(bf16).reshape(2, 128, 1024)
        in_maps.append(
            {
                "ones128": np.ones((1, 128), bf16),
                "xq": xq_,
                "xk": xk_,
                "xv": xv_,
                "wq": wq4,
                "wk": wk4,
                "wv": wv4,
                "wo": wo4,
                "bqd": bqd,
                "bkd": bkd,
                "bvr": bvr,
                "tri": tri01,
            }
        )
    return in_maps


def run_cores(in_maps, trace=False, trace_kwargs=None, repeat=1):
    """Compile + run the SPMD program on cores 0-7, return BassKernelResults."""
    from concourse.bass_utils import run_bass_kernel_spmd

    nc = _get_program(repeat)
    kwargs = {}
    if trace:
        kwargs["trace"] = True
        if trace_kwargs:
            kwargs["trace_kwargs"] = trace_kwargs
    return run_bass_kernel_spmd(nc, in_maps, core_ids=list(range(N_CORES)), **kwargs)


def kernel(query, key, value, mask, Wq, bq, Wk, bk, Wv, bv, Wo, bo, _trace=False):
    in_maps = _host_inputs(query, key, value, Wq, bq, Wk, bk, Wv, bv, Wo)
    res = run_cores(in_maps, trace=_trace)
    bo = np.asarray(bo, dtype=np.float32)
    out = np.zeros((B, S, D), dtype=np.float32)
    for core in range(N_CORES):
        b = core // 4
        out[b] += np.asarray(res.results[core]["out"], dtype=np.float32)
    out += bo[None, None, :]
    kernel.last_results = res
    return out

